# revision 1
# baseline (speedup 1.0000x reference)
"""Trainium2 Bass kernel for nn_CTR_27754078666791 (batched Sinkhorn OT loss).

Reference semantics: 200-iteration Sinkhorn with a convergence check at
t = 0, 50, 100, 150 that freezes the iterates once
    max_b |sum_k u_new*Kv - sum_k a| <= 5e-3.
Because u_new = a/(Kv+eps), the checked quantity is a/(Kv+eps)*Kv ~ a up to
f32 rounding (~1e-4), so the check passes at t=0 for any inputs: the loop
always freezes after ONE Sinkhorn iteration from the uniform init
u0 = 1/K, v0 = 1/V.  The computation therefore reduces to:

    E[v,k]  = exp(-alpha*M[v,k])                  (K_mat transposed)
    s[v]    = sum_k E[v,k] / K                     (= K^T u0, batch-indep)
    v1[b,v] = b[b,v] / (s[v] + eps)
    Kv1     = v1 @ E          [B,K]
    G       = v1 @ (E*M)      [B,K]
    u1      = a / (Kv1 + eps)
    loss    = mean_b sum_k u1[b,k] * G[b,k]

Distribution: shard V=5000 across 8 cores (625 rows of M / cols of b each).
Each core reads only its M/b shard (~0.8 MB), produces partial Kv1_c / G_c
[B,K] sums; the tiny [64,256] partials are summed on host ("final mean
all-reduce"), where u1 and the loss are formed.  This moves 6.4 MB total
across 8 cores instead of replicating the 5 MB cost matrix per core.
"""

import numpy as np

# Problem constants (hardcoded per harness contract).
B = 64
K = 256
V = 5000
NCORES = 8
VC = V // NCORES  # 625 rows of M per core
P = 125           # partition rows per tile
NT = VC // P      # 5 tiles per core
ALPHA = 20.0
EPS = 1e-16

_CACHE = {}


def _build_nc():
    from concourse import bacc, mybir, tile, masks

    f32 = mybir.dt.float32
    Act = mybir.ActivationFunctionType

    nc = bacc.Bacc(
        "TRN2",
        debug=False,
        enable_asserts=False,
        num_devices=NCORES,
    )
    m_d = nc.dram_tensor("m_sh", [VC, K], f32, kind="ExternalInput").ap()
    b_d = nc.dram_tensor("b_sh", [B, VC], f32, kind="ExternalInput").ap()
    o_d = nc.dram_tensor("out", [B, 2 * K], f32, kind="ExternalOutput").ap()

    with tile.TileContext(nc) as tc:
        with (
            tc.tile_pool(name="const", bufs=1) as constp,
            tc.tile_pool(name="bsb", bufs=1) as bpool,
            tc.tile_pool(name="mt", bufs=3) as mpool,
            tc.tile_pool(name="et", bufs=3) as epool,
            tc.tile_pool(name="ct", bufs=2) as cpool,
            tc.tile_pool(name="v1", bufs=3) as vpool,
            tc.tile_pool(name="sc", bufs=NT) as spool,
            tc.tile_pool(name="osb", bufs=1) as opool,
            tc.tile_pool(name="pt", bufs=2, space="PSUM") as ptp,
            tc.tile_pool(name="pacc", bufs=1, space="PSUM") as paccp,
        ):
            ident = constp.tile([B, B], f32, tag="ident")
            masks.make_identity(nc, ident[:])

            b_sb = bpool.tile([B, VC], f32, tag="bsb")
            nc.sync.dma_start(out=b_sb[:], in_=b_d)

            psum_kv = paccp.tile([B, K], f32, tag="kv")
            psum_g = paccp.tile([B, K], f32, tag="g")

            for i in range(NT):
                mt = mpool.tile([P, K], f32, tag="m")
                nc.sync.dma_start(out=mt[:], in_=m_d[i * P : (i + 1) * P, :])

                # E = exp(-alpha*M); s = rowsum(E) fused into the same ACT op.
                et = epool.tile([P, K], f32, tag="e")
                s = spool.tile([P, 1], f32, tag="s")
                nc.scalar.activation(
                    et[:], mt[:], Act.Exp, scale=-ALPHA, accum_out=s[:]
                )

                # r = 1 / (s/K + eps)   (reference: v = b/(K^T u0 + eps))
                t = spool.tile([P, 1], f32, tag="t")
                nc.scalar.activation(t[:], s[:], Act.Copy, bias=EPS, scale=1.0 / K)
                r = spool.tile([P, 1], f32, tag="r")
                nc.vector.reciprocal(r[:], t[:])

                # bT tile via PE transpose, then v1T = r * bT (PSUM->SBUF).
                pt = ptp.tile([P, B], f32, tag="pt")
                nc.tensor.transpose(pt[:], b_sb[:, i * P : (i + 1) * P], ident[:])
                v1t = vpool.tile([P, B], f32, tag="v1t")
                nc.scalar.activation(v1t[:], pt[:], Act.Copy, scale=r[:])

                # Kv1 += v1T.T @ E ; G += v1T.T @ (E*M)
                nc.tensor.matmul(
                    psum_kv[:], v1t[:], et[:], start=(i == 0), stop=(i == NT - 1)
                )
                ct = cpool.tile([P, K], f32, tag="c")
                nc.vector.tensor_mul(ct[:], et[:], mt[:])
                nc.tensor.matmul(
                    psum_g[:], v1t[:], ct[:], start=(i == 0), stop=(i == NT - 1)
                )

            out_sb = opool.tile([B, 2 * K], f32, tag="osb")
            nc.vector.tensor_copy(out_sb[:, 0:K], psum_kv[:])
            nc.vector.tensor_copy(out_sb[:, K : 2 * K], psum_g[:])
            nc.sync.dma_start(out=o_d, in_=out_sb[:])

    nc.compile()
    return nc


def _get_nc():
    if "nc" not in _CACHE:
        _CACHE["nc"] = _build_nc()
    return _CACHE["nc"]


def _make_in_maps(b, M):
    in_maps = []
    for c in range(NCORES):
        lo, hi = c * VC, (c + 1) * VC
        in_maps.append(
            {
                "m_sh": np.ascontiguousarray(M[lo:hi, :], dtype=np.float32),
                "b_sh": np.ascontiguousarray(b[:, lo:hi], dtype=np.float32),
            }
        )
    return in_maps


def run_on_hw(a, b, M, trace=False):
    """Returns (loss, BassKernelResults)."""
    from concourse import bass_utils

    nc = _get_nc()
    res = bass_utils.run_bass_kernel_spmd(
        nc,
        _make_in_maps(np.asarray(b), np.asarray(M)),
        core_ids=list(range(NCORES)),
        trace=trace,
    )
    outs = [res.results[c]["out"] for c in range(NCORES)]
    acc = np.sum(np.stack(outs, axis=0), axis=0)  # [B, 2K]
    kv1 = acc[:, :K]
    g = acc[:, K:]
    u1 = np.asarray(a, dtype=np.float32) / (kv1 + np.float32(EPS))
    loss = np.float32(np.mean(np.sum(u1 * g, axis=1)))
    return np.asarray(loss), res


def kernel(a, b, M):
    loss, _ = run_on_hw(a, b, M, trace=False)
    return loss


# revision 2
# speedup vs baseline: 1.1877x; 1.1877x over previous
"""Trainium2 Bass kernel for nn_CTR_27754078666791 (batched Sinkhorn OT loss).

Reference semantics: 200-iteration Sinkhorn with a convergence check at
t = 0, 50, 100, 150 that freezes the iterates once
    max_b |sum_k u_new*Kv - sum_k a| <= 5e-3.
Because u_new = a/(Kv+eps), the checked quantity is a/(Kv+eps)*Kv ~ a up to
f32 rounding (~1e-4), so the check passes at t=0 for any inputs: the loop
always freezes after ONE Sinkhorn iteration from the uniform init
u0 = 1/K, v0 = 1/V.  The computation therefore reduces to:

    E[v,k]  = exp(-alpha*M[v,k])                  (K_mat transposed)
    s[v]    = sum_k E[v,k] / K                     (= K^T u0, batch-indep)
    v1[b,v] = b[b,v] / (s[v] + eps)
    Kv1     = v1 @ E          [B,K]
    G       = v1 @ (E*M)      [B,K]
    u1      = a / (Kv1 + eps)
    loss    = mean_b sum_k u1[b,k] * G[b,k]

Distribution: shard V=5000 across 8 cores (625 rows of M / cols of b each).
Each core reads only its M/b shard (~0.8 MB), produces partial Kv1_c / G_c
[B,K] sums; the tiny [64,256] partials are summed on host (the final mean
all-reduce), where u1 and the loss are formed.

Device kernel per core (Tile):
  - b arrives pre-transposed from host as bf16 [VC,B] (v on partitions), so
    no on-device transposes are needed.
  - per 125-row tile: ACT computes E=exp(-20*M) (bf16) with fused row-sum
    accumulator; DVE computes r=1/(s/K+eps) and v1T=r*bT (bf16) and
    C=E*M (bf16) packed next to E in one [125,512] tile; PE runs ONE bf16
    matmul per tile accumulating [Kv1|G] into a single [64,512] PSUM bank.
  - input DMAs are split across both HWDGE rings (SP + ACT) for overlap.
"""

import numpy as np

# Problem constants (hardcoded per harness contract).
B = 64
K = 256
V = 5000
NCORES = 8
VC = V // NCORES  # 625 rows of M per core
P = 125           # partition rows per tile
NT = VC // P      # 5 tiles per core
ALPHA = 20.0
EPS = 1e-16

_CACHE = {}


def _build_nc():
    from concourse import bacc, mybir, tile

    f32 = mybir.dt.float32
    bf16 = mybir.dt.bfloat16
    Act = mybir.ActivationFunctionType

    nc = bacc.Bacc(
        "TRN2",
        debug=False,
        enable_asserts=False,
        num_devices=NCORES,
    )
    m_d = nc.dram_tensor("m_sh", [VC, K], f32, kind="ExternalInput").ap()
    bt_d = nc.dram_tensor("bt_sh", [VC, B], bf16, kind="ExternalInput").ap()
    o_d = nc.dram_tensor("out", [B, 2 * K], f32, kind="ExternalOutput").ap()

    with tile.TileContext(nc) as tc:
        with (
            tc.tile_pool(name="mt", bufs=NT) as mpool,
            tc.tile_pool(name="bt", bufs=NT) as btpool,
            tc.tile_pool(name="ec", bufs=NT) as ecpool,
            tc.tile_pool(name="v1", bufs=NT) as vpool,
            tc.tile_pool(name="sc", bufs=NT) as spool,
            tc.tile_pool(name="osb", bufs=1) as opool,
            tc.tile_pool(name="pacc", bufs=1, space="PSUM") as paccp,
        ):
            psum = paccp.tile([B, 2 * K], f32, tag="acc")

            # Input DMAs, alternating between the two HWDGE rings.
            mts = []
            bts = []
            for i in range(NT):
                mt = mpool.tile([P, K], f32, tag="m")
                bt = btpool.tile([P, B], bf16, tag="bt")
                eng = nc.sync if i % 2 == 0 else nc.scalar
                beng = nc.scalar if i % 2 == 0 else nc.sync
                eng.dma_start(out=mt[:], in_=m_d[i * P : (i + 1) * P, :])
                beng.dma_start(out=bt[:], in_=bt_d[i * P : (i + 1) * P, :])
                mts.append(mt)
                bts.append(bt)

            for i in range(NT):
                mt, bt = mts[i], bts[i]
                # [E | C] packed tile: E = exp(-alpha*M) with fused row-sum
                # into s; C = E*M.
                ec = ecpool.tile([P, 2 * K], bf16, tag="ec")
                s = spool.tile([P, 1], f32, tag="s")
                nc.scalar.activation(
                    ec[:, 0:K], mt[:], Act.Exp, scale=-ALPHA, accum_out=s[:]
                )
                nc.vector.tensor_mul(ec[:, K : 2 * K], ec[:, 0:K], mt[:])

                # r = 1 / (s/K + eps); v1T = r * bT (bf16).
                t = spool.tile([P, 1], f32, tag="t")
                nc.scalar.activation(t[:], s[:], Act.Copy, bias=EPS, scale=1.0 / K)
                r = spool.tile([P, 1], f32, tag="r")
                nc.vector.reciprocal(r[:], t[:])
                v1t = vpool.tile([P, B], bf16, tag="v1t")
                nc.vector.tensor_scalar_mul(v1t[:], bt[:], r[:])

                # [Kv1 | G] += v1T.T @ [E | C]
                nc.tensor.matmul(
                    psum[:], v1t[:], ec[:], start=(i == 0), stop=(i == NT - 1)
                )

            out_sb = opool.tile([B, 2 * K], f32, tag="osb")
            nc.vector.tensor_copy(out_sb[:], psum[:])
            nc.sync.dma_start(out=o_d, in_=out_sb[:])

    nc.compile()
    return nc


def _get_nc():
    if "nc" not in _CACHE:
        _CACHE["nc"] = _build_nc()
    return _CACHE["nc"]


def _make_in_maps(b, M):
    import ml_dtypes

    bt = np.ascontiguousarray(
        np.asarray(b, dtype=np.float32).T.astype(ml_dtypes.bfloat16)
    )  # [V, B] bf16
    M = np.asarray(M, dtype=np.float32)
    in_maps = []
    for c in range(NCORES):
        lo, hi = c * VC, (c + 1) * VC
        in_maps.append(
            {
                "m_sh": np.ascontiguousarray(M[lo:hi, :]),
                "bt_sh": np.ascontiguousarray(bt[lo:hi, :]),
            }
        )
    return in_maps


def run_on_hw(a, b, M, trace=False):
    """Returns (loss, BassKernelResults)."""
    from concourse import bass_utils

    nc = _get_nc()
    res = bass_utils.run_bass_kernel_spmd(
        nc,
        _make_in_maps(b, M),
        core_ids=list(range(NCORES)),
        trace=trace,
    )
    outs = [res.results[c]["out"] for c in range(NCORES)]
    acc = np.sum(np.stack(outs, axis=0), axis=0)  # [B, 2K]
    kv1 = acc[:, :K]
    g = acc[:, K:]
    u1 = np.asarray(a, dtype=np.float32) / (kv1 + np.float32(EPS))
    loss = np.float32(np.mean(np.sum(u1 * g, axis=1)))
    return np.asarray(loss), res


def kernel(a, b, M):
    loss, _ = run_on_hw(a, b, M, trace=False)
    return loss
